# revision 1
# baseline (speedup 1.0000x reference)
"""Biaffine edge attention on 8 Trainium2 NeuronCores.

Math (per batch b):
    out[i,o] = head[i,:] @ U @ dep[o,:] + head[i,:]@wh + dep[o,:]@wd + b
with head/dep [S=2048, D=256], U [D,D], edge_W = [wh | wd] (each [D]).

Sharding: pure data-parallel over batch B=8 -> one batch per core,
U / edge_W / edge_b replicated. No collectives.

Per-core kernel:
    ATf[e,i] = sum_d U[d,e] * headT[d,i] + wd[e]      (the dep-side rank-1
               term ds[o] rides the e-contraction for free)
    hs[i]    = sum_d head[i,d] * wh[d]  + b           (DVE mul+reduce;
               per-partition bias in the epilogue)
    out[i,o] = sum_e ATf[e,i] * depT[e,o]  + hs[i]
head and dep are transposed on-chip with PE transposes (batched into
[128,512] PSUM collect tiles). Matmuls run as float32r (1 cycle/row for
moving dim >= 256 vs 4 for strict fp32 => this is what makes the problem
memory- instead of compute-bound). FP32r matmul inputs must be rounded to
f32r by a compute op, so matmul-feeding SBUF tiles are float32r-typed and
written by DVE/ACT copies, never directly by DMA.

DMA sizing: inputs load as [128,1024] group tiles (4 row-blocks per DMA via
a 3D access pattern), outputs store as [128,1024] tiles -- keeps the SP
sequencer's per-DMA dispatch cost (~0.65us) well below the ~60us of data
movement.
"""

import contextlib

import numpy as np

import concourse.bass as bass
import concourse.tile as tile
from concourse import bacc, mybir
from concourse.bass_utils import run_bass_kernel_spmd

B, S, D = 8, 2048, 256
P = 128          # partitions
OC = 512         # matmul output free-dim chunk (one PSUM bank of fp32)
GB = 4           # row-blocks per input load group
NG = S // (P * GB)   # 4 load groups per input
NI = S // P      # 16 row blocks
NO = S // OC     # 4 output column chunks
ND = D // P      # 2 contraction chunks
F32 = mybir.dt.float32
F32R = mybir.dt.float32r


def build_nc(reps=1):
    """reps>1 wraps the body in a HW For_i loop -- used only for timing."""
    nc = bacc.Bacc("TRN2", target_bir_lowering=False, debug=False, num_devices=B)

    head_d = nc.dram_tensor("head", [S, D], F32, kind="ExternalInput")
    dep_d = nc.dram_tensor("dep", [S, D], F32, kind="ExternalInput")
    u_d = nc.dram_tensor("U", [D, D], F32, kind="ExternalInput")
    whr_d = nc.dram_tensor("wh_rep", [P, GB * D], F32, kind="ExternalInput")
    wdT_d = nc.dram_tensor("wdT", [P, ND], F32, kind="ExternalInput")
    b128_d = nc.dram_tensor("b128", [P, 1], F32, kind="ExternalInput")
    eye_d = nc.dram_tensor("eye", [P, P], F32, kind="ExternalInput")
    out_d = nc.dram_tensor("out", [S, S], F32, kind="ExternalOutput")

    Ident = mybir.ActivationFunctionType.Identity

    with tile.TileContext(nc) as tc:
        with (
            tc.tile_pool(name="const", bufs=1) as cpool,
            tc.tile_pool(name="persist", bufs=1) as ppool,
            tc.tile_pool(name="stage", bufs=3) as stage,
            tc.tile_pool(name="ttrp", bufs=2) as ttrp,
            tc.tile_pool(name="outbuf", bufs=3) as outbuf,
            tc.tile_pool(name="ps_t", bufs=2, space=bass.MemorySpace.PSUM) as ps_t,
            tc.tile_pool(name="ps_mm", bufs=6, space=bass.MemorySpace.PSUM) as ps_mm,
        ):
            # ---- constants ----
            eye = cpool.tile([P, P], F32, name="eye", tag="eye")
            nc.sync.dma_start(eye[:], eye_d[:])
            b128 = cpool.tile([P, 1], F32, name="b128", tag="b128")
            nc.sync.dma_start(b128[:], b128_d[:])
            wh_rep = cpool.tile([P, GB * D], F32, name="wh_rep", tag="wh_rep")
            nc.sync.dma_start(wh_rep[:], whr_d[:])
            wdT = cpool.tile([P, ND], F32, name="wdT", tag="wdT")
            nc.sync.dma_start(wdT[:], wdT_d[:])
            u_sb = []
            for dc in range(ND):
                u_stg = cpool.tile([P, D], F32, name=f"ustg{dc}", tag=f"ustg{dc}")
                nc.sync.dma_start(u_stg[:], u_d[dc * P:(dc + 1) * P, :])
                u_t = cpool.tile([P, D], F32R, name=f"u{dc}", tag=f"u{dc}")
                nc.vector.tensor_copy(u_t[:], u_stg[:])
                u_sb.append(u_t)

            # ---- persistent SBUF tensors ----
            headT = [ppool.tile([P, S], F32R, name=f"headT{dc}", tag=f"headT{dc}")
                     for dc in range(ND)]
            depT = [ppool.tile([P, S], F32R, name=f"depT{dc}", tag=f"depT{dc}")
                    for dc in range(ND)]
            atf = [ppool.tile([P, S], F32R, name=f"atf{eb}", tag=f"atf{eb}")
                   for eb in range(ND)]
            hs_col = ppool.tile([P, NI], F32, name="hs_col", tag="hs_col")
            hs_colb = ppool.tile([P, NI], F32, name="hs_colb", tag="hs_colb")

            def load_group(src_dram, g):
                # [128, GB*D]: free = (block j, d); one DMA, 3D src pattern
                nat = stage.tile([P, GB * D], F32, name="nat", tag="nat")
                src = src_dram[g * GB * P:(g + 1) * GB * P, :]
                src3 = src.rearrange("(j p) d -> p j d", p=P)
                nc.sync.dma_start(nat[:].rearrange("p (j d) -> p j d", d=D), src3)
                return nat

            def transpose_group(nat, dstT, g, eng_off):
                # 8 PE transposes -> two [128,512] PSUM collect tiles -> 2 copies
                for dc in range(ND):
                    pst = ps_t.tile([P, GB * P], F32, name="pst", tag="pst")
                    for j in range(GB):
                        nc.tensor.transpose(
                            pst[:, j * P:(j + 1) * P],
                            nat[:, j * D + dc * P: j * D + dc * P + P],
                            eye[:],
                        )
                    dst = dstT[dc][:, g * GB * P:(g + 1) * GB * P]
                    if (g * ND + dc + eng_off) % 2 == 0:
                        nc.vector.tensor_copy(dst, pst[:])
                    else:
                        nc.scalar.copy(dst, pst[:])

            def body():
                # ---- interleaved loads / transposes / hs / AT ----
                for g in range(NG):
                    nat_h = load_group(head_d, g)
                    nat_p = load_group(dep_d, g)
                    transpose_group(nat_h, headT, g, 0)
                    # hs for this group's 4 blocks: mul + blockwise reduce
                    ttr = ttrp.tile([P, GB * D], F32, name="ttr", tag="ttr")
                    nc.vector.tensor_mul(ttr[:], nat_h[:], wh_rep[:])
                    nc.vector.reduce_sum(
                        hs_col[:, g * GB:(g + 1) * GB],
                        ttr[:].rearrange("p (j d) -> p j d", d=D),
                        axis=mybir.AxisListType.X,
                    )
                    nc.scalar.activation(
                        hs_colb[:, g * GB:(g + 1) * GB],
                        hs_col[:, g * GB:(g + 1) * GB], Ident,
                        bias=b128[:, 0:1],
                    )
                    transpose_group(nat_p, depT, g, 1)
                    # ATf chunk ic=g (headT[:, g*512:(g+1)*512] just written)
                    for eb in range(ND):
                        pa = ps_mm.tile([P, OC], F32, name="psmm", tag="psmm")
                        for dc in range(ND):
                            nc.tensor.matmul(
                                pa[:],
                                u_sb[dc][:, eb * P:(eb + 1) * P],
                                headT[dc][:, g * OC:(g + 1) * OC],
                                start=(dc == 0),
                                stop=(dc == ND - 1),
                            )
                        nc.scalar.activation(
                            atf[eb][:, g * OC:(g + 1) * OC], pa[:], Ident,
                            bias=wdT[:, eb:eb + 1],
                        )

                # ---- big matmul + fused epilogue, full-row out tiles ----
                for ib in range(NI):
                    ot = outbuf.tile([P, S], F32, name="ot", tag="ot")
                    for oc in range(NO):
                        po = ps_mm.tile([P, OC], F32, name="psmm", tag="psmm")
                        for eb in range(ND):
                            nc.tensor.matmul(
                                po[:],
                                atf[eb][:, ib * P:(ib + 1) * P],
                                depT[eb][:, oc * OC:(oc + 1) * OC],
                                start=(eb == 0),
                                stop=(eb == ND - 1),
                            )
                        dst = ot[:, oc * OC:(oc + 1) * OC]
                        if (ib + oc) % 2 == 0:
                            nc.scalar.activation(
                                dst, po[:], Ident, bias=hs_colb[:, ib:ib + 1]
                            )
                        else:
                            nc.vector.tensor_scalar_add(
                                dst, po[:], hs_colb[:, ib:ib + 1]
                            )
                    nc.sync.dma_start(out_d[ib * P:(ib + 1) * P, :], ot[:])

            if reps > 1:
                with tc.For_i(0, reps, 1):
                    body()
            else:
                body()

    nc.finalize()
    return nc


_NC_CACHE = {}


def _get_nc(reps=1):
    if reps not in _NC_CACHE:
        _NC_CACHE[reps] = build_nc(reps)
    return _NC_CACHE[reps]


def make_in_maps(head, dep, edge_U, edge_W, edge_b):
    head = np.ascontiguousarray(np.asarray(head, dtype=np.float32))
    dep = np.ascontiguousarray(np.asarray(dep, dtype=np.float32))
    u = np.ascontiguousarray(np.asarray(edge_U, dtype=np.float32))
    w = np.asarray(edge_W, dtype=np.float32).reshape(-1)
    wh, wd = w[:D], w[D:]
    wh_rep = np.ascontiguousarray(np.tile(wh[None, :], (P, GB)))
    wdT = np.ascontiguousarray(wd.reshape(ND, P).T)
    b128 = np.full((P, 1), float(np.asarray(edge_b).reshape(-1)[0]), np.float32)
    eye = np.eye(P, dtype=np.float32)
    return [
        {
            "head": head[b], "dep": dep[b], "U": u,
            "wh_rep": wh_rep, "wdT": wdT, "b128": b128, "eye": eye,
        }
        for b in range(B)
    ]


def kernel(head, dep, edge_U, edge_W, edge_b):
    nc = _get_nc()
    in_maps = make_in_maps(head, dep, edge_U, edge_W, edge_b)
    res = run_bass_kernel_spmd(nc, in_maps, core_ids=list(range(B)))
    return np.stack([res.results[b]["out"] for b in range(B)], axis=0)



# revision 6
# speedup vs baseline: 1.2244x; 1.2244x over previous
"""Biaffine edge attention on 8 Trainium2 NeuronCores.

Math (per batch b):
    out[i,o] = head[i,:] @ U @ dep[o,:] + head[i,:]@wh + dep[o,:]@wd + b
with head/dep [S=2048, D=256], U [D,D], edge_W = [wh | wd] (each [D]).

Sharding: pure data-parallel over batch B=8 -> one batch per core,
U / edge_W / edge_b replicated. No collectives.

Per-core kernel:
    ATf[e,i] = sum_d U[d,e] * headT[d,i] + wd[e]      (dep-side rank-1 term
               rides the e-contraction for free)
    hs[i]    = sum_d head[i,d] * wh[d]  + b           (GpSimd mul+reduce)
    out[i,o] = sum_e ATf[e,i] * depT[e,o]  + hs[i]

The schedule is built around keeping the HBM pipe (~358 GB/s/core) busy
continuously -- 21.4 MB of mandatory traffic = ~59 us floor:
  * ONE packed const DMA (eye|U|wh|wdT|b) instead of seven.
  * All input loads issued back-to-back at program start on the scalar
    (ACT) HWDGE ring; every group tile persists in SBUF so no load ever
    waits on a buffer.  Load order: consts, dep g0..g3, head g0..g3 --
    dep first so the full depT (needed by every out tile) is ready ~15us.
  * Out stores go on the sync (SP) HWDGE ring -- independent FIFO from
    the loads, so the first store issues the moment out-tile 0's epilogue
    lands (~17us) while trailing head loads still stream on the ACT ring.
  * Engine split: PE transposes + matmuls (~48us total, the secondary
    critical path), DVE collects + half the epilogue, ACT load triggers +
    ATf bias + other half of the epilogue, GpSimd the hs reduction.
Matmuls run as float32r (1 cycle/row for moving dim >= 256).  FP32r
matmul inputs must be produced by a compute op, so matmul-feeding SBUF
tiles are float32r-typed and written by DVE/ACT copies, never by DMA.
"""

import numpy as np

import concourse.bass as bass
import concourse.tile as tile
from concourse import bacc, mybir
from concourse.bass_utils import run_bass_kernel_spmd

B, S, D = 8, 2048, 256
P = 128          # partitions
OC = 512         # matmul output free-dim chunk (one PSUM bank of fp32)
GB = 4           # row-blocks per input load group
NG = S // (P * GB)   # 4 load groups per input
NI = S // P      # 16 row blocks
NO = S // OC     # 4 output column chunks
ND = D // P      # 2 contraction chunks
F32 = mybir.dt.float32
F32R = mybir.dt.float32r

# packed const layout: eye | U0 | U1 | wh128 | wdT | b128
C_EYE, C_U0, C_U1, C_WH, C_WDT, C_B = 0, P, P + D, P + 2 * D, P + 2 * D + D, P + 3 * D + ND
C_TOT = C_B + 1


def build_nc(reps=1):
    nc = bacc.Bacc("TRN2", target_bir_lowering=False, debug=False, num_devices=B)

    head_d = nc.dram_tensor("head", [S, D], F32, kind="ExternalInput")
    dep_d = nc.dram_tensor("dep", [S, D], F32, kind="ExternalInput")
    cst_d = nc.dram_tensor("cpack", [P, C_TOT], F32, kind="ExternalInput")
    out_d = nc.dram_tensor("out", [S, S], F32, kind="ExternalOutput")

    Ident = mybir.ActivationFunctionType.Identity

    with tile.TileContext(nc) as tc:
        with (
            tc.tile_pool(name="const", bufs=1) as cpool,
            tc.tile_pool(name="persist", bufs=1) as ppool,
            tc.tile_pool(name="ttrp", bufs=2) as ttrp,
            tc.tile_pool(name="outbuf", bufs=6) as outbuf,
            tc.tile_pool(name="ps_t", bufs=2, space=bass.MemorySpace.PSUM) as ps_t,
            tc.tile_pool(name="ps_mm", bufs=6, space=bass.MemorySpace.PSUM) as ps_mm,
        ):
            # ---- all loads issued upfront on the ACT HWDGE ring ----
            cst = cpool.tile([P, C_TOT], F32, name="cst", tag="cst")
            nc.scalar.dma_start(cst[:], cst_d[:])

            nat_d, nat_h = [], []
            for g in range(NG):
                t = ppool.tile([P, GB * D], F32, name=f"natd{g}", tag=f"natd{g}")
                src = dep_d[g * GB * P:(g + 1) * GB * P, :]
                nc.scalar.dma_start(
                    t[:].rearrange("p (j d) -> p j d", d=D),
                    src.rearrange("(j p) d -> p j d", p=P),
                )
                nat_d.append(t)
            for g in range(NG):
                t = ppool.tile([P, GB * D], F32, name=f"nath{g}", tag=f"nath{g}")
                src = head_d[g * GB * P:(g + 1) * GB * P, :]
                nc.scalar.dma_start(
                    t[:].rearrange("p (j d) -> p j d", d=D),
                    src.rearrange("(j p) d -> p j d", p=P),
                )
                nat_h.append(t)

            # ---- f32r copies of U (DVE, right after cst lands) ----
            u_sb = []
            for dc in range(ND):
                u_t = cpool.tile([P, D], F32R, name=f"u{dc}", tag=f"u{dc}")
                nc.vector.tensor_copy(u_t[:], cst[:, C_U0 + dc * D:C_U0 + (dc + 1) * D])
                u_sb.append(u_t)
            eye = cst[:, C_EYE:C_EYE + P]

            # ---- persistent SBUF tensors ----
            headT = [ppool.tile([P, S], F32R, name=f"headT{dc}", tag=f"headT{dc}")
                     for dc in range(ND)]
            depT = [ppool.tile([P, S], F32R, name=f"depT{dc}", tag=f"depT{dc}")
                    for dc in range(ND)]
            atf = [ppool.tile([P, S], F32R, name=f"atf{eb}", tag=f"atf{eb}")
                   for eb in range(ND)]
            hs_col = ppool.tile([P, NI], F32, name="hs_col", tag="hs_col")
            hs_colb = ppool.tile([P, NI], F32, name="hs_colb", tag="hs_colb")

            def transpose_group(nat, dstT, g):
                # 8 PE transposes -> two [128,512] PSUM collect tiles -> DVE
                for dc in range(ND):
                    pst = ps_t.tile([P, GB * P], F32, name="pst", tag="pst")
                    for j in range(GB):
                        nc.tensor.transpose(
                            pst[:, j * P:(j + 1) * P],
                            nat[:, j * D + dc * P: j * D + dc * P + P],
                            eye,
                        )
                    nc.vector.tensor_copy(dstT[dc][:, g * GB * P:(g + 1) * GB * P], pst[:])

            def out_tile(ib):
                ot = outbuf.tile([P, S], F32, name="ot", tag="ot")
                for oc in range(NO):
                    po = ps_mm.tile([P, OC], F32, name="psmm", tag="psmm")
                    for eb in range(ND):
                        nc.tensor.matmul(
                            po[:],
                            atf[eb][:, ib * P:(ib + 1) * P],
                            depT[eb][:, oc * OC:(oc + 1) * OC],
                            start=(eb == 0),
                            stop=(eb == ND - 1),
                        )
                    dst = ot[:, oc * OC:(oc + 1) * OC]
                    if (ib + oc) % 2 == 0:
                        nc.scalar.activation(
                            dst, po[:], Ident, bias=hs_colb[:, ib:ib + 1]
                        )
                    else:
                        nc.vector.tensor_scalar_add(
                            dst, po[:], hs_colb[:, ib:ib + 1]
                        )
                nc.sync.dma_start(out_d[ib * P:(ib + 1) * P, :], ot[:])

            def body():
                # dep groups as they arrive
                for g in range(NG):
                    transpose_group(nat_d[g], depT, g)
                # head groups: transpose + hs + AT, then the 4 out row-blocks
                # this group unlocks
                for g in range(NG):
                    transpose_group(nat_h[g], headT, g)
                    # hs for this group's 4 blocks on GpSimd
                    ttr = ttrp.tile([P, GB * D], F32, name="ttr", tag="ttr")
                    for j in range(GB):
                        nc.gpsimd.tensor_mul(
                            ttr[:, j * D:(j + 1) * D],
                            nat_h[g][:, j * D:(j + 1) * D],
                            cst[:, C_WH:C_WH + D],
                        )
                    nc.vector.reduce_sum(
                        hs_col[:, g * GB:(g + 1) * GB],
                        ttr[:].rearrange("p (j d) -> p j d", d=D),
                        axis=mybir.AxisListType.X,
                    )
                    nc.gpsimd.tensor_scalar_add(
                        hs_colb[:, g * GB:(g + 1) * GB],
                        hs_col[:, g * GB:(g + 1) * GB],
                        cst[:, C_B:C_B + 1],
                    )
                    # ATf chunk g (headT[:, g*512:(g+1)*512] just written)
                    for eb in range(ND):
                        pa = ps_mm.tile([P, OC], F32, name="psmm", tag="psmm")
                        for dc in range(ND):
                            nc.tensor.matmul(
                                pa[:],
                                u_sb[dc][:, eb * P:(eb + 1) * P],
                                headT[dc][:, g * OC:(g + 1) * OC],
                                start=(dc == 0),
                                stop=(dc == ND - 1),
                            )
                        nc.scalar.activation(
                            atf[eb][:, g * OC:(g + 1) * OC], pa[:], Ident,
                            bias=cst[:, C_WDT + eb:C_WDT + eb + 1],
                        )
                    for ib in range(g * GB, (g + 1) * GB):
                        out_tile(ib)

            if reps > 1:
                with tc.For_i(0, reps, 1):
                    body()
            else:
                body()

    nc.finalize()
    return nc


_NC_CACHE = {}


def _get_nc(reps=1):
    if reps not in _NC_CACHE:
        _NC_CACHE[reps] = build_nc(reps)
    return _NC_CACHE[reps]


def make_in_maps(head, dep, edge_U, edge_W, edge_b):
    head = np.ascontiguousarray(np.asarray(head, dtype=np.float32))
    dep = np.ascontiguousarray(np.asarray(dep, dtype=np.float32))
    u = np.asarray(edge_U, dtype=np.float32)
    w = np.asarray(edge_W, dtype=np.float32).reshape(-1)
    wh, wd = w[:D], w[D:]
    bval = float(np.asarray(edge_b).reshape(-1)[0])

    cpack = np.zeros((P, C_TOT), dtype=np.float32)
    cpack[:, C_EYE:C_EYE + P] = np.eye(P, dtype=np.float32)
    cpack[:, C_U0:C_U0 + D] = u[0:P, :]
    cpack[:, C_U1:C_U1 + D] = u[P:2 * P, :]
    cpack[:, C_WH:C_WH + D] = np.tile(wh[None, :], (P, 1))
    cpack[:, C_WDT:C_WDT + ND] = wd.reshape(ND, P).T
    cpack[:, C_B] = bval
    cpack = np.ascontiguousarray(cpack)

    return [
        {"head": head[b], "dep": dep[b], "cpack": cpack}
        for b in range(B)
    ]


def kernel(head, dep, edge_U, edge_W, edge_b):
    nc = _get_nc()
    in_maps = make_in_maps(head, dep, edge_U, edge_W, edge_b)
    res = run_bass_kernel_spmd(nc, in_maps, core_ids=list(range(B)))
    return np.stack([res.results[b]["out"] for b in range(B)], axis=0)
